# revision 18
# baseline (speedup 1.0000x reference)
"""Trainium2 Bass kernel for ConditionedLSTMTransition.

2-layer LSTM, T=128 steps, B=512, D=512, data-parallel over batch on 8 cores
(64 rows/core). Everything on-device is feature-major ([feature, batch]) so
the recurrence needs no transposes:

  gates.T tile [128, 64] = W.T_slice.T @ h.T_ktile   (weights stationary, fp16)

Per step, per layer: 64 h-matmuls (16 m-tiles x 4 k-tiles, N=64 streaming
cols) accumulate onto PSUM pre-filled with (static-conditioning + bias) via
an identity matmul plus the u-projection (both emitted during the previous
step's elementwise phase, PE otherwise idle then). The LSTM cell runs fp32
on ACT (sigmoid/tanh) + DVE (mult/add) with per-gate PSUM banks so each
gate's activation starts as soon as its matmuls finish. h is carried fp16
(written straight into the Z output buffer); c stays fp32. Host folds dt
into W_u/D, combines layer-1's Wih+Whh (input==hidden there), permutes
gates to (g,i,f,o), and augments the static projection with a ones-row so
biases ride the same matmul.

All inputs ship as ONE packed [128, F] fp16 tensor (single DMA) and outputs
as one Z (fp16, host-upcast) + one Y (fp32) DMA — this walrus build allows
only ~2 sync commands per instruction, so DMA-queue fan-out must stay tiny.
"""

import numpy as np

import concourse.bass as bass
import concourse.mybir as mybir
import concourse.tile as tile
from concourse.bass_utils import run_bass_kernel_spmd

F16 = mybir.dt.float16
F32 = mybir.dt.float32

T, B, D, DSTAT, UDIM, NOBS = 128, 512, 512, 256, 128, 40
NCORES = 8
BSH = B // NCORES          # 64 batch rows per core
MT = 16                    # m-tiles of 128 gate-features (4*D/128)
KT = 4                     # k-tiles of 128 over D
KS = 3                     # k-tiles over augmented static dim (256+1 -> 384)

GATE_ORDER = ("g", "i", "f", "o")
GATE_M0 = {"g": 0, "i": 4, "f": 8, "o": 12}


def _pack_layout(nsteps):
    sizes = [
        ("ut", nsteps * BSH),
        ("wu", 4 * D),
        ("whh0", KT * 4 * D),
        ("w1", KT * 4 * D),
        ("wsa", KS * 4 * D),
        ("ct", KT * NOBS),
        ("dtm", NOBS),
        ("init1", MT * BSH),
        ("ident", 128),
        ("zsa", KS * BSH),
        ("zd", KT * BSH),
    ]
    off, lay = 0, {}
    for nm, sz in sizes:
        lay[nm] = (off, sz)
        off += sz
    return lay, off


def _out_dma_sems(nc):
    sems = set()
    for blk in nc.m.functions[0].blocks:
        for ins in blk.instructions:
            if type(ins).__name__ != "InstDMACopy":
                continue
            txt = str(ins.concise()) if hasattr(ins, "concise") else ""
            if "zy" in txt:
                for u in (ins.sync_info.on_update if ins.sync_info else []):
                    sems.add(u.ant_name or "")
    return sems


def _fix_self_waits(nc):
    """This walrus build allows only ~2 sync commands per instruction. Tile
    sometimes emits an extra wait on the instruction's own engine semaphore
    (bank-overlap / WAR edges); those are redundant by program order
    (engines complete in-order; the engine sem is only incremented by this
    engine's instructions), so drop them."""
    pref = {
        mybir.EngineType.Activation: "Activation_",
        mybir.EngineType.DVE: "DVE_",
        mybir.EngineType.PE: "PE_",
        mybir.EngineType.Pool: "Pool_",
    }
    for blk in nc.m.functions[0].blocks:
        for ins in blk.instructions:
            si = ins.sync_info
            if not si or not si.on_wait or len(si.on_wait) <= 1:
                continue
            p = pref.get(ins.engine)
            if p is None and type(ins).__name__ != "InstDrain":
                continue
            kept = [w for w in si.on_wait
                    if p is None or not (w.ant_name or "").startswith(p)]
            if len(kept) != len(si.on_wait):
                ins.sync_info = mybir.SyncInfo(
                    on_wait=kept, on_update=list(si.on_update))
            if type(ins).__name__ == "InstDrain" and len(kept) > 2:
                # keep only the output-DMA queue waits: engine quiesce is
                # enforced by the barrier that follows, and the input pack
                # DMA completed transitively (all compute consumed it).
                kept = [w for w in kept
                        if (w.ant_name or "") in _out_dma_sems(nc)]
                ins.sync_info = mybir.SyncInfo(
                    on_wait=kept, on_update=list(si.on_update))
            if len(kept) > 1 and type(ins).__name__ != "InstDrain":
                raise AssertionError(
                    f"{type(ins).__name__} {ins.name} still has waits "
                    f"{[(w.ant_name, w.wait_value) for w in kept]}")


def _build(nsteps=T):
    lay, packf = _pack_layout(nsteps)
    nc = bass.Bass()

    pack_d = nc.dram_tensor("pack", [128, packf], F16, kind="ExternalInput")
    # single output: [z slots 1..T | y] packed per partition, fp16
    outf = nsteps * KT * BSH + nsteps * BSH
    zy_d = nc.dram_tensor("zy", [128, outf], F16, kind="ExternalOutput")

    Sig = mybir.ActivationFunctionType.Sigmoid
    Tan = mybir.ActivationFunctionType.Tanh
    Mult = mybir.AluOpType.mult
    Add = mybir.AluOpType.add

    with tile.TileContext(nc) as tc:
        with (
            tc.tile_pool(name="weights", bufs=1) as wpool,
            tc.tile_pool(name="work", bufs=2) as wk,
            tc.tile_pool(name="psum0", bufs=1, space=bass.MemorySpace.PSUM) as ps0,
            tc.tile_pool(name="psum1", bufs=1, space=bass.MemorySpace.PSUM) as ps1,
        ):
            pack = wpool.tile([128, packf], F16, name="pack")
            outbuf = wpool.tile([128, (nsteps + 1) * KT * BSH + nsteps * BSH],
                                F16, name="outbuf")
            zb = outbuf[:, :(nsteps + 1) * KT * BSH].rearrange(
                "p (t k b) -> p t k b", t=nsteps + 1, k=KT)
            y_sb = outbuf[:NOBS, (nsteps + 1) * KT * BSH:].rearrange(
                "p (t b) -> p t b", t=nsteps)
            c1_sb = wpool.tile([128, KT, BSH], F32, name="c1")
            c2_sb = wpool.tile([128, KT, BSH], F32, name="c2")

            def pv(nm):
                o, s = lay[nm]
                return pack[:, o:o + s]

            ut_sb = pv("ut").rearrange("p (t b) -> p t b", t=nsteps)
            wu_sb = pv("wu")
            whh0_sb = pv("whh0").rearrange("p (k n) -> p k n", k=KT)
            w1_sb = pv("w1").rearrange("p (k n) -> p k n", k=KT)
            wsa_sb = pv("wsa").rearrange("p (k n) -> p k n", k=KS)
            ct_sb = pv("ct").rearrange("p (k n) -> p k n", k=KT)
            dtm_sb = pv("dtm")
            init1_sb = pv("init1").rearrange("p (m b) -> p m b", m=MT)
            ident_sb = pv("ident")
            zsa_sb = pv("zsa").rearrange("p (k b) -> p k b", k=KS)
            zd_sb = pv("zd").rearrange("p (k b) -> p k b", k=KT)

            nc.sync.dma_start(pack[:], pack_d[:])

            nc.vector.memset(c1_sb[:], 0.0)
            nc.vector.memset(c2_sb[:], 0.0)
            # y region: partitions >= NOBS are never written; zero for the DMA
            nc.vector.memset(outbuf[:, (nsteps + 1) * KT * BSH:], 0.0)
            # initial h (= z_dyn.T) into zb slot 0
            nc.vector.tensor_copy(zb[:, 0, :, :], zd_sb)

            # warmup: ACT table loads attach to dep-light instructions
            warm = wpool.tile([128, 1], F32, name="warm")
            nc.vector.memset(warm[:], 0.0)
            nc.scalar.activation(warm[:], warm[:], Sig)
            nc.scalar.activation(warm[:], warm[:], Tan)

            mm = nc.tensor.matmul

            def gate_tiles(pool, layer):
                return {gn: pool.tile([128, 4, BSH], F32, tag=f"g{layer}_{gn}",
                                      name=f"g{layer}_{gn}")
                        for gn in GATE_ORDER}

            # ---- one-time INIT0 = (Ws_aug @ zs_aug).T = (S + b0).T ---------
            init0_sb = wpool.tile([128, MT, BSH], F16, name="init0")
            g0 = gate_tiles(ps0, 0)
            for gn in GATE_ORDER:
                for j in range(4):
                    m = GATE_M0[gn] + j
                    for k in range(KS):
                        mm(g0[gn][:, j, :], wsa_sb[:, k, m * 128:(m + 1) * 128],
                           zsa_sb[:, k, :],
                           start=(j == 0 and k == 0),
                           stop=(j == 3 and k == KS - 1),
                           skip_group_check=True)
            for gn in GATE_ORDER:
                nc.scalar.copy(init0_sb[:, GATE_M0[gn]:GATE_M0[gn] + 4, :],
                               g0[gn][:])

            def prefill(g0t, t):
                """init0 + u-projection into step t's gates0 psum."""
                for gn in GATE_ORDER:
                    for j in range(4):
                        m = GATE_M0[gn] + j
                        mm(g0t[gn][:, j, :], ident_sb, init0_sb[:, m, :],
                           start=(j == 0), stop=False, skip_group_check=True)
                        mm(g0t[gn][:, j, :], wu_sb[:, m * 128:(m + 1) * 128],
                           ut_sb[:, t, :], start=False, stop=False,
                           skip_group_check=True)

            def layer_mms(g_t, w_sb, h_f16):
                for gn in GATE_ORDER:
                    for j in range(4):
                        m = GATE_M0[gn] + j
                        for k in range(KT):
                            mm(g_t[gn][:, j, :],
                               w_sb[:, k, m * 128:(m + 1) * 128],
                               h_f16[:, k, :],
                               start=False, stop=(j == 3 and k == KT - 1),
                               skip_group_check=True)

            def cell_ew(g_t, c_sb, h_out):
                tg = wk.tile([128, 4, BSH], F32, tag="tg", name="tg")
                si = wk.tile([128, 4, BSH], F32, tag="si", name="si")
                sf = wk.tile([128, 4, BSH], F32, tag="sf", name="sf")
                so = wk.tile([128, 4, BSH], F32, tag="so", name="so")
                t1 = wk.tile([128, 4, BSH], F32, tag="t1", name="t1")
                t2 = wk.tile([128, 4, BSH], F32, tag="t2", name="t2")
                tc_ = wk.tile([128, 4, BSH], F32, tag="tc", name="tc")
                act = nc.scalar.activation
                act(tg[:], g_t["g"][:], Tan)
                act(si[:], g_t["i"][:], Sig)
                act(sf[:], g_t["f"][:], Sig)
                act(so[:], g_t["o"][:], Sig)
                nc.vector.tensor_tensor(t2[:], si[:], tg[:], Mult)
                nc.vector.tensor_tensor(t1[:], sf[:], c_sb[:], Mult)
                nc.vector.tensor_tensor(c_sb[:], t1[:], t2[:], Add)
                act(tc_[:], c_sb[:], Tan)
                nc.vector.tensor_tensor(h_out, so[:], tc_[:], Mult)

            def head(t):
                """y[t] from zb slot t+1 (h2 of step t); D-part first so the
                C-part matmuls carry only the DVE wait. t=-1 is a throwaway
                (result overwritten) used to seed the DVE<-PE clock."""
                yp = ps0.tile([NOBS, BSH], F32, tag="g0_o", name="yp")
                mm(yp[:], dtm_sb, ut_sb[:, max(t, 0), :],
                   start=True, stop=False, skip_group_check=True)
                for k in range(KT):
                    mm(yp[:], ct_sb[:, k, :], zb[:, t + 1, k, :],
                       start=False, stop=(k == KT - 1), skip_group_check=True)
                nc.vector.tensor_copy(y_sb[:, max(t, 0), :], yp[:])

            # prologue: pre-fill step-0 gates0
            prefill(g0, 0)

            for t in range(nsteps):
                # layer-1 bias pre-fill (b1 broadcast via identity matmul)
                g1 = gate_tiles(ps1, 1)
                for gn in GATE_ORDER:
                    for j in range(4):
                        m = GATE_M0[gn] + j
                        mm(g1[gn][:, j, :], ident_sb, init1_sb[:, m, :],
                           start=(j == 0), stop=False, skip_group_check=True)
                # layer 0
                layer_mms(g0, whh0_sb, zb[:, t, :, :])
                h1 = wk.tile([128, KT, BSH], F16, tag="h1", name="h1")
                cell_ew(g0, c1_sb, h1[:])
                # layer 1
                layer_mms(g1, w1_sb, h1[:])
                if t > 0:
                    head(t - 1)
                else:
                    head(-1)  # throwaway: syncs DVE's PE-clock
                cell_ew(g1, c2_sb, zb[:, t + 1, :, :])
                # pre-fill next step's gates0 while ew1 runs
                if t + 1 < nsteps:
                    g0 = gate_tiles(ps0, 0)
                    prefill(g0, t + 1)

            head(nsteps - 1)
            nc.sync.dma_start(zy_d[:], outbuf[:, KT * BSH:])

    _fix_self_waits(nc)
    return nc


def _prep_inputs(z_dyn, z_static, dt, U, Wih0, Whh0, bih0, bhh0,
                 Wih1, Whh1, bih1, bhh1, C, D_, nsteps=T):
    """Host-side marshaling: shard over batch, fold dt, permute gates to
    (g,i,f,o), transpose to feature-major, cast fp16, pack per core."""
    f16 = np.float16
    lay, packf = _pack_layout(nsteps)
    dtv = np.float32(dt.reshape(-1)[0])

    def perm_rows(W):
        i, f, g, o = np.split(W, 4, axis=0)
        return np.concatenate([g, i, f, o], axis=0)

    Wu = perm_rows(Wih0[:, :UDIM] * dtv)
    Ws = perm_rows(Wih0[:, UDIM:])
    b0 = perm_rows((bih0 + bhh0).reshape(4 * D, 1))[:, 0]
    Whh0p = perm_rows(Whh0)
    W1p = perm_rows(Wih1 + Whh1)
    b1 = perm_rows((bih1 + bhh1).reshape(4 * D, 1))[:, 0]

    WsA = np.zeros((4 * D, KS * 128), np.float32)
    WsA[:, :DSTAT] = Ws
    WsA[:, DSTAT] = b0

    wu = Wu.T                                                     # [128, 2048]
    whh0 = Whh0p.T.reshape(KT, 128, 4 * D).transpose(1, 0, 2).reshape(128, -1)
    w1 = W1p.T.reshape(KT, 128, 4 * D).transpose(1, 0, 2).reshape(128, -1)
    wsa = WsA.T.reshape(KS, 128, 4 * D).transpose(1, 0, 2).reshape(128, -1)
    ct = C.T.reshape(KT, 128, NOBS).transpose(1, 0, 2).reshape(128, -1)
    dtm = (D_ * dtv).T                                            # [128, 40]
    init1 = np.broadcast_to(
        b1.reshape(MT, 128).T[:, :, None], (128, MT, BSH)).reshape(128, -1)
    ident = np.eye(128, dtype=np.float32)

    shared = [("wu", wu), ("whh0", whh0), ("w1", w1), ("wsa", wsa),
              ("ct", ct), ("dtm", dtm), ("init1", init1), ("ident", ident)]

    in_maps = []
    for c in range(NCORES):
        sl = slice(c * BSH, (c + 1) * BSH)
        ut = U[:nsteps, sl, :].transpose(2, 0, 1).reshape(128, -1)
        zsa = np.zeros((KS * 128, BSH), np.float32)
        zsa[:DSTAT] = z_static[sl].T
        zsa[DSTAT] = 1.0
        zsa = zsa.reshape(KS, 128, BSH).transpose(1, 0, 2).reshape(128, -1)
        zd = z_dyn[sl].T.reshape(KT, 128, BSH).transpose(1, 0, 2).reshape(128, -1)
        pk = np.empty((128, packf), f16)
        for nm, arr in shared + [("ut", ut), ("zsa", zsa), ("zd", zd)]:
            o, s = lay[nm]
            assert arr.shape == (128, s), (nm, arr.shape, s)
            pk[:, o:o + s] = arr.astype(f16)
        in_maps.append({"pack": pk})
    return in_maps


_CACHE = {}


def _run(inputs, nsteps=T, trace=False):
    if nsteps not in _CACHE:
        _CACHE[nsteps] = _build(nsteps)
    nc = _CACHE[nsteps]
    in_maps = _prep_inputs(**inputs, nsteps=nsteps)
    res = run_bass_kernel_spmd(nc, in_maps, list(range(NCORES)), trace=trace)
    Zs, Ys = [], []
    nz = nsteps * KT * BSH
    for r in res.results:
        zy = r["zy"].astype(np.float32)    # [128, nz + nsteps*BSH]
        zc = zy[:, :nz].reshape(128, nsteps, KT, BSH)
        # Z[t, b, 128k+p] = zc[p, t, k, b]
        Zs.append(zc.transpose(1, 3, 2, 0).reshape(nsteps, BSH, D))
        yc = zy[:NOBS, nz:].reshape(NOBS, nsteps, BSH)
        Ys.append(yc.transpose(1, 2, 0))
    return (np.concatenate(Zs, axis=1), np.concatenate(Ys, axis=1)), res


def kernel(z_dyn, z_static, dt, U, Wih0, Whh0, bih0, bhh0,
           Wih1, Whh1, bih1, bhh1, C, D, **_ignored):
    inputs = dict(z_dyn=np.asarray(z_dyn, np.float32),
                  z_static=np.asarray(z_static, np.float32),
                  dt=np.asarray(dt, np.float32),
                  U=np.asarray(U, np.float32),
                  Wih0=np.asarray(Wih0, np.float32),
                  Whh0=np.asarray(Whh0, np.float32),
                  bih0=np.asarray(bih0, np.float32),
                  bhh0=np.asarray(bhh0, np.float32),
                  Wih1=np.asarray(Wih1, np.float32),
                  Whh1=np.asarray(Whh1, np.float32),
                  bih1=np.asarray(bih1, np.float32),
                  bhh1=np.asarray(bhh1, np.float32),
                  C=np.asarray(C, np.float32),
                  D_=np.asarray(D, np.float32))
    (Z, Y), _ = _run(inputs)
    return Z, Y
